# revision 24
# baseline (speedup 1.0000x reference)
"""Cross-attention Bass/Tile kernel for Trainium2, data-parallel over batch on
8 NeuronCores.

Reference computation (per batch b):
    Q = tokens @ Wq            [T, EMB]
    K = context @ Wk           [S, EMB]
    V = context @ Wv           [S, HID]
    scores = Q @ K.T / sqrt(EMB)
    attn = softmax(scores, axis=-1)
    out = attn @ V             [T, HID]

Shapes: B=8, T=4096, S=1024, HID=512, EMB=512, CTX=768 (fp32).

Design notes:
- One batch per core (B == n_cores == 8), no collectives.
- Weight fold: scores = tokens @ (Wq @ K^T), so the per-token Q projection is
  eliminated. Phase A computes CAT = Wq @ K^T [h, s] once per batch (K^T =
  Wk^T @ context^T); phase B contracts tokens^T against CAT directly.
- Scores are computed TRANSPOSED, [s, t], so the exp(P^T) tiles in SBUF feed
  the attn@V matmul directly as the stationary operand — no transpose of the
  4M-element P.
- Softmax skips the max-subtraction: scores/sqrt(EMB) are ~N(0,1) here (randn
  inputs, 1/sqrt(fan_in)-scaled weights), so exp stays comfortably in range;
  1/sqrt(EMB) is folded into the ACT exp scale.
- All matmul operands are bf16 (PSUM accumulation stays fp32). fp32->bf16
  input casts run on DVE; exp runs on the Scalar engine.
- ALL transposes (context^T, Wq^T, tokens^T) run on the DMA xbar
  (dma_start_transpose, 2-byte dtype, SBUF->SBUF): the tensor engine executes
  nothing but matmuls. For out shape [128, M, 128] the xbar writes
  out[p, m, c] = in[c, m*128 + p].
- Row sums of exp are folded into the attn@V matmul: V is augmented with a
  ones column (layout [V[:, :256] | 1 | V[:, 256:] | 1]) and each t-block
  runs 2x N=257 matmuls per s-block into two PSUM banks. Column 256 of the
  first bank is sum_s exp = the softmax denominator, in exactly the layout
  ([t-part, 1]) needed for the per-partition reciprocal + scale.
- Token loads are software-pipelined one chunk ahead (DMA -> DVE cast ->
  xbar transpose), with chunk 0 hoisted into phase A.
- Input DMAs ride the two HWDGE queues in priority order (sync: ctx q0, Wk,
  ctx q2, Wv; scalar: ctx q1, Wq, ctx q3) — SWDGE (gpsimd) descriptor
  generation is far too slow for bulk input loads.
"""

import math

import numpy as np

from concourse import bacc, mybir, tile
from concourse.bass_utils import run_bass_kernel_spmd

B, T, S = 8, 4096, 1024
HID, EMB, CTX = 512, 512, 768
P = 128  # partitions
TC = 512  # t-chunk processed per phase-B iteration
N_TC = T // TC  # 8
F32 = mybir.dt.float32
BF16 = mybir.dt.bfloat16

HC = HID // P  # 4 h chunks
EC = EMB // P  # 4 e chunks
CC = CTX // P  # 6 c chunks
SB = S // P    # 8 s blocks
TB = TC // P   # 4 t blocks per chunk
HH = HID // 2  # 256, half of the output features per augmented-V matmul
NQ = 4         # context DMA quarters
QS = SB // NQ  # 2 s-blocks per quarter


def build():
    nc = bacc.Bacc("TRN2", target_bir_lowering=False, debug=False)

    tokens = nc.declare_dram_parameter("tokens", [T, HID], F32, isOutput=False)
    context = nc.declare_dram_parameter("context", [S, CTX], F32, isOutput=False)
    wq = nc.declare_dram_parameter("Wq", [HID, EMB], F32, isOutput=False)
    wk = nc.declare_dram_parameter("Wk", [CTX, EMB], F32, isOutput=False)
    wv = nc.declare_dram_parameter("Wv", [CTX, HID], F32, isOutput=False)
    out = nc.declare_dram_parameter("out", [T, HID], F32, isOutput=True)

    inv_sqrt_emb = 1.0 / math.sqrt(EMB)

    with tile.TileContext(nc) as tc:
        with (
            tc.tile_pool(name="persist", bufs=1) as persist,
            tc.tile_pool(name="pb_tok", bufs=2) as pb_tok,
            tc.tile_pool(name="pb_tokb", bufs=2) as pb_tokb,
            tc.tile_pool(name="pb_tokt", bufs=2) as pb_tokt,
            tc.tile_pool(name="pb_pt", bufs=16) as pb_pt,
            tc.tile_pool(name="pb_small", bufs=8) as pb_small,
            tc.tile_pool(name="pb_out", bufs=2) as pb_out,
        ):
            # CAT = Wq @ K^T [h, s], built once per batch
            cat_sb = persist.tile([P, HC, S], BF16)
            # V augmented with ones columns: [V[:, 0:256] | 1 | V[:, 256:512] | 1]
            v_aug = persist.tile([P, SB, 2 * (HH + 1)], BF16)
            nc.vector.memset(v_aug, 1.0)

            def emit_tok_load(ti):
                """DMA (scalar q) -> bf16 cast (DVE) -> xbar transpose (sync q).
                tokt[p, tb, hc, t2] = tokens[ti*TC + tb*128 + t2, hc*128 + p]"""
                tok_nat = pb_tok.tile([P, TB, HID], F32, tag="tok")
                nc.scalar.dma_start(
                    out=tok_nat,
                    in_=tokens[ti * TC:(ti + 1) * TC, :].rearrange(
                        "(n p) h -> p n h", p=P
                    ),
                )
                tok_bf = pb_tokb.tile([P, TB, HID], BF16, tag="tokb")
                nc.vector.tensor_copy(out=tok_bf, in_=tok_nat)
                tokt = pb_tokt.tile([P, TB, HC, P], BF16, tag="tokt")
                nc.sync.dma_start_transpose(out=tokt, in_=tok_bf)
                return tokt

            # ---- Phase A: context -> context^T -> K^T -> CAT, V ----
            with (
                tc.tile_pool(name="pa_sbuf", bufs=4) as pa_sbuf,
                tc.tile_pool(name="pa_wst", bufs=1) as pa_wst,
                tc.tile_pool(name="pa_wb", bufs=1) as pa_wb,
                tc.tile_pool(name="pa_ctxt", bufs=1) as pa_ctxt,
                tc.tile_pool(name="pa_psum", bufs=4, space="PSUM") as pa_psum,
            ):
                # Input DMAs: the SDMA engines drain ALL in-flight transfers
                # round-robin, so "priority" = don't trigger a transfer until
                # it's actually next in criticality. The gating is done by
                # placing later triggers behind dependent ops on the same
                # engine queue. First wave: ctx q0/q1 + Wk (K^T s-half 0).
                ctx_nat = [None] * NQ
                for q in range(NQ):
                    ctx_nat[q] = pa_sbuf.tile(
                        [P, QS, CTX], F32, tag="ctxn", name=f"ctx_nat{q}"
                    )

                def dma_ctx(q, eng):
                    eng.dma_start(
                        out=ctx_nat[q],
                        in_=context[q * (S // NQ):(q + 1) * (S // NQ), :].rearrange(
                            "(n p) c -> p n c", p=P
                        ),
                    )

                wk_st = pa_wst.tile([P, CC, EMB], F32, tag="wkst")
                wq_st = pa_wst.tile([P, HC, EMB], F32, tag="wqst")
                wv_st = pa_wst.tile([P, CC, HID], F32, tag="wvst")
                dma_ctx(0, nc.sync)
                dma_ctx(1, nc.scalar)
                nc.sync.dma_start(out=wk_st, in_=wk.rearrange("(c p) e -> p c e", p=P))
                nc.scalar.dma_start(out=wq_st, in_=wq.rearrange("(h p) e -> p h e", p=P))

                # ctx^T [c, s] (contiguous s) via xbar, one trigger per
                # s-block: ctxt[p, cc, sb*128+s2] = ctx[sb*128+s2, cc*128+p].
                # The s-contiguous layout keeps the K^T moving operand's
                # stride at 1024B-aligned full rate (a [128-elem x stride-768]
                # pattern measured 2x slower).
                ctxt = pa_ctxt.tile([P, CC, S], BF16)
                ctx_bf = []

                def cast_and_transpose_ctx(q):
                    cb = pa_sbuf.tile(
                        [P, QS, CTX], BF16, tag="ctxb", name=f"ctx_bf{q}"
                    )
                    nc.vector.tensor_copy(out=cb, in_=ctx_nat[q])
                    for j in range(QS):
                        sb = q * QS + j
                        nc.sync.dma_start_transpose(
                            out=ctxt[:, :, sb * P:(sb + 1) * P], in_=cb[:, j, :]
                        )
                    ctx_bf.append(cb)

                cast_and_transpose_ctx(0)
                # second DMA wave rides behind the first xbars on sync
                dma_ctx(2, nc.sync)
                dma_ctx(3, nc.sync)
                cast_and_transpose_ctx(1)

                wk_sb = pa_wb.tile([P, CC, EMB], BF16)
                nc.scalar.activation(
                    out=wk_sb, in_=wk_st, func=mybir.ActivationFunctionType.Copy
                )
                wq_sb = pa_wb.tile([P, HC, EMB], BF16)
                nc.scalar.activation(
                    out=wq_sb, in_=wq_st, func=mybir.ActivationFunctionType.Copy
                )
                # Wq^T via xbar: wqt[p, hc, ec, c] = Wq^T[ec*128+p, hc*128+c]
                wqt = pa_wb.tile([P, HC, EC, P], BF16)
                nc.sync.dma_start_transpose(out=wqt, in_=wq_sb)
                # third wave: Wv (needed for V after CAT s-half 0)
                nc.sync.dma_start(out=wv_st, in_=wv.rearrange("(c p) h -> p c h", p=P))

                kt = pa_ctxt.tile([P, EC, S], BF16)  # K^T [e, s]

                def mm_kt(sh):
                    # K^T[e, s-half] accumulation over c chunks
                    for ec in range(EC):
                        pk = pa_psum.tile([P, 512], F32, tag="pa_kv")
                        for cc in range(CC):
                            nc.tensor.matmul(
                                pk,
                                wk_sb[:, cc, ec * P:(ec + 1) * P],
                                ctxt[:, cc, sh * 512:(sh + 1) * 512],
                                start=(cc == 0),
                                stop=(cc == CC - 1),
                            )
                        nc.vector.tensor_copy(
                            out=kt[:, ec, sh * 512:(sh + 1) * 512], in_=pk
                        )

                def mm_cat(sh):
                    # CAT[h, s-half] = Wq @ K^T, accumulation over e chunks
                    for hc in range(HC):
                        pc = pa_psum.tile([P, 512], F32, tag="pa_kv")
                        for ec in range(EC):
                            nc.tensor.matmul(
                                pc,
                                wqt[:, hc, ec, :],
                                kt[:, ec, sh * 512:(sh + 1) * 512],
                                start=(ec == 0),
                                stop=(ec == EC - 1),
                            )
                        nc.vector.tensor_copy(
                            out=cat_sb[:, hc, sh * 512:(sh + 1) * 512], in_=pc
                        )

                def mm_v(sb_lo, sb_hi):
                    # V[s, h] accumulation over c chunks; results go into the
                    # two halves of the ones-augmented layout.
                    for sb in range(sb_lo, sb_hi):
                        pv = pa_psum.tile([P, 512], F32, tag="pa_kv")
                        for cc in range(CC):
                            nc.tensor.matmul(
                                pv,
                                ctxt[:, cc, sb * P:(sb + 1) * P],
                                wv_sb[:, cc, :],
                                start=(cc == 0),
                                stop=(cc == CC - 1),
                            )
                        nc.vector.tensor_copy(
                            out=v_aug[:, sb, 0:HH], in_=pv[:, 0:HH]
                        )
                        nc.vector.tensor_copy(
                            out=v_aug[:, sb, HH + 1:2 * HH + 1], in_=pv[:, HH:HID]
                        )

                mm_kt(0)
                # JIT: ctx half-1 casts + xbar (DVE queue served half-0 first)
                cast_and_transpose_ctx(2)
                cast_and_transpose_ctx(3)
                wv_sb = pa_wb.tile([P, CC, HID], BF16)
                nc.scalar.activation(
                    out=wv_sb, in_=wv_st, func=mybir.ActivationFunctionType.Copy
                )
                # chunk-0 token pipeline, hoisted so its DMA/cast/xbar land
                # well before phase B starts
                tokts = {0: emit_tok_load(0)}
                mm_cat(0)
                mm_v(0, 4)
                mm_kt(1)
                mm_cat(1)
                mm_v(4, 8)

            # ---- Phase B: stream over t chunks, software-pipelined by one
            # chunk (attn of chunk i-1 runs between scores of chunk i and
            # chunk i+1, so the PE never waits for the trailing exp tiles) ----
            with (
                tc.tile_pool(name="ps_s", bufs=4, space="PSUM") as ps_s,
                tc.tile_pool(name="ps_ctx", bufs=2, space="PSUM") as ps_ctx,
            ):
                def emit_scores(ti):
                    # scores^T [s, t] = CAT^T @ tokens^T -> exp -> P^T tiles
                    tokt = tokts.pop(ti)
                    if ti + 1 < N_TC:
                        tokts[ti + 1] = emit_tok_load(ti + 1)
                    pts = []
                    for sb in range(SB):
                        pscore = ps_s.tile([P, TC], F32, tag="s")
                        for hc in range(HC):
                            nc.tensor.matmul(
                                pscore,
                                cat_sb[:, hc, sb * P:(sb + 1) * P],
                                tokt[:, :, hc, :],
                                start=(hc == 0),
                                stop=(hc == HC - 1),
                            )
                        pt_tile = pb_pt.tile([P, TC], BF16, tag="pt")
                        nc.scalar.activation(
                            out=pt_tile,
                            in_=pscore,
                            func=mybir.ActivationFunctionType.Exp,
                            scale=inv_sqrt_emb,
                        )
                        pts.append(pt_tile)
                    return pts

                def emit_attn(ti, pts):
                    # attn@V with the ones-augmented V: two N=257 matmuls per
                    # s-block into two PSUM banks; column 256 of bank a is the
                    # softmax denominator in [t-part, 1] layout.
                    o_all = pb_out.tile([P, TB, HID], F32, tag="out")
                    for tb in range(TB):
                        pca = ps_ctx.tile([P, HH + 1], F32, tag="ctxa")
                        pcb = ps_ctx.tile([P, HH + 1], F32, tag="ctxb")
                        for sb in range(SB):
                            st = pts[sb][:, tb * P:(tb + 1) * P]
                            nc.tensor.matmul(
                                pca,
                                st,
                                v_aug[:, sb, 0:HH + 1],
                                start=(sb == 0),
                                stop=(sb == SB - 1),
                            )
                            nc.tensor.matmul(
                                pcb,
                                st,
                                v_aug[:, sb, HH + 1:2 * (HH + 1)],
                                start=(sb == 0),
                                stop=(sb == SB - 1),
                            )
                        rec = pb_small.tile([P, 1], F32, tag="rec")
                        nc.vector.reciprocal(out=rec, in_=pca[:, HH:HH + 1])
                        nc.vector.tensor_scalar_mul(
                            o_all[:, tb, 0:HH], pca[:, 0:HH], rec
                        )
                        nc.vector.tensor_scalar_mul(
                            o_all[:, tb, HH:HID], pcb[:, 0:HH], rec
                        )
                        nc.sync.dma_start(
                            out=out[ti * TC + tb * P:ti * TC + (tb + 1) * P, :],
                            in_=o_all[:, tb, :],
                        )

                prev_pts = None
                for ti in range(N_TC):
                    pts = emit_scores(ti)
                    if prev_pts is not None:
                        emit_attn(ti - 1, prev_pts)
                    prev_pts = pts
                emit_attn(N_TC - 1, prev_pts)

    nc.compile()
    return nc


_NC_CACHE = None


def _get_nc():
    global _NC_CACHE
    if _NC_CACHE is None:
        _NC_CACHE = build()
    return _NC_CACHE


def kernel(tokens, context, Wq, Wk, Wv):
    tokens = np.ascontiguousarray(np.asarray(tokens, dtype=np.float32))
    context = np.ascontiguousarray(np.asarray(context, dtype=np.float32))
    Wq = np.ascontiguousarray(np.asarray(Wq, dtype=np.float32))
    Wk = np.ascontiguousarray(np.asarray(Wk, dtype=np.float32))
    Wv = np.ascontiguousarray(np.asarray(Wv, dtype=np.float32))

    nc = _get_nc()
    in_maps = [
        {
            "tokens": tokens[b],
            "context": context[b],
            "Wq": Wq,
            "Wk": Wk,
            "Wv": Wv,
        }
        for b in range(B)
    ]
    res = run_bass_kernel_spmd(nc, in_maps, core_ids=list(range(B)))
    return np.stack([res.results[b]["out"] for b in range(B)], axis=0)


# revision 26
# speedup vs baseline: 1.1194x; 1.1194x over previous
"""Cross-attention Bass/Tile kernel for Trainium2, data-parallel over batch on
8 NeuronCores.

Reference computation (per batch b):
    Q = tokens @ Wq            [T, EMB]
    K = context @ Wk           [S, EMB]
    V = context @ Wv           [S, HID]
    scores = Q @ K.T / sqrt(EMB)
    attn = softmax(scores, axis=-1)
    out = attn @ V             [T, HID]

Shapes: B=8, T=4096, S=1024, HID=512, EMB=512, CTX=768 (fp32).

Design notes:
- One batch per core (B == n_cores == 8), no collectives.
- Weight fold: scores = tokens @ (Wq @ K^T), so the per-token Q projection is
  eliminated. Phase A computes CAT = Wq @ K^T [h, s] once per batch (K^T =
  Wk^T @ context^T); phase B contracts tokens^T against CAT directly.
- Scores are computed TRANSPOSED, [s, t], so the exp(P^T) tiles in SBUF feed
  the attn@V matmul directly as the stationary operand — no transpose of the
  4M-element P.
- Softmax skips the max-subtraction: scores/sqrt(EMB) are ~N(0,1) here; the
  1/sqrt(EMB) is folded into the ACT exp scale.
- All matmul operands are bf16 (PSUM accumulation stays fp32). fp32->bf16
  input casts run on DVE (ctx, tokens) and Scalar (weights); exp runs on the
  Scalar engine.
- tokens^T and Wq^T are produced by single-trigger DMA xbar transposes
  (2-byte SBUF->SBUF; for out [128, M, 128]: out[p, m, c] = in[c, m*128+p]).
  context^T stays on PE transpose-mode: the xbar rings are FIFO with only
  ~155 GB/s each, so phase-A xbar transfers would queue behind the bulk
  input DMAs on the head critical path.
- Moving operands must keep >=1024B-aligned strides: a [128-elem x
  stride-768-elem] bf16 pattern runs at half rate; stride-512-elem (1024B)
  runs at full rate (tokt layout relies on this).
- Row sums of exp are folded into the attn@V matmul: V is augmented with a
  ones column (layout [V[:, :256] | 1 | V[:, 256:] | 1]) and each t-block
  runs 2x N=257 matmuls per s-block into two PSUM banks. Column 256 of the
  first bank is sum_s exp = the softmax denominator, in exactly the layout
  ([t-part, 1]) needed for the per-partition reciprocal + scale.
- Token loads are software-pipelined one chunk ahead (DMA on the scalar
  ring -> DVE cast -> xbar transpose, with the xbar trigger emitted after
  the chunk's exps so it never blocks them); chunk 0 is hoisted into phase
  A, and chunk 0's scores for s-half 0/1 are interleaved into phase A's
  DMA shadow.
- Input DMAs ride the two HWDGE rings; the SDMA engines drain rings at
  ~155 GB/s each, FIFO per ring, so the critical tensors lead each ring:
  sync: [ctx q0, Wk, ctx q2, out-stores]; scalar: [ctx q1, Wq, ctx q3, Wv,
  tokens].
"""

import math

import numpy as np

from concourse import bacc, mybir, tile
from concourse.bass_utils import run_bass_kernel_spmd
from concourse.masks import make_identity

B, T, S = 8, 4096, 1024
HID, EMB, CTX = 512, 512, 768
P = 128  # partitions
TC = 512  # t-chunk processed per phase-B iteration
N_TC = T // TC  # 8
F32 = mybir.dt.float32
BF16 = mybir.dt.bfloat16

HC = HID // P  # 4 h chunks
EC = EMB // P  # 4 e chunks
CC = CTX // P  # 6 c chunks
SB = S // P    # 8 s blocks
TB = TC // P   # 4 t blocks per chunk
HH = HID // 2  # 256, half of the output features per augmented-V matmul
NQ = 4         # context DMA quarters
QS = SB // NQ  # 2 s-blocks per quarter


def build():
    nc = bacc.Bacc("TRN2", target_bir_lowering=False, debug=False)

    tokens = nc.declare_dram_parameter("tokens", [T, HID], F32, isOutput=False)
    context = nc.declare_dram_parameter("context", [S, CTX], F32, isOutput=False)
    wq = nc.declare_dram_parameter("Wq", [HID, EMB], F32, isOutput=False)
    wk = nc.declare_dram_parameter("Wk", [CTX, EMB], F32, isOutput=False)
    wv = nc.declare_dram_parameter("Wv", [CTX, HID], F32, isOutput=False)
    out = nc.declare_dram_parameter("out", [T, HID], F32, isOutput=True)

    inv_sqrt_emb = 1.0 / math.sqrt(EMB)

    with tile.TileContext(nc) as tc:
        with (
            tc.tile_pool(name="persist", bufs=1) as persist,
            tc.tile_pool(name="pb_tok", bufs=2) as pb_tok,
            tc.tile_pool(name="pb_tokb", bufs=2) as pb_tokb,
            tc.tile_pool(name="pb_tokt", bufs=2) as pb_tokt,
            tc.tile_pool(name="pb_pt", bufs=16) as pb_pt,
            tc.tile_pool(name="pb_small", bufs=8) as pb_small,
            tc.tile_pool(name="pb_out", bufs=2) as pb_out,
            tc.tile_pool(name="ps_s", bufs=4, space="PSUM") as ps_s,
        ):
            ident = persist.tile([P, P], BF16)
            make_identity(nc, ident)

            # CAT = Wq @ K^T [h, s], built once per batch
            cat_sb = persist.tile([P, HC, S], BF16)
            # V augmented with ones columns: [V[:, 0:256] | 1 | V[:, 256:512] | 1]
            v_aug = persist.tile([P, SB, 2 * (HH + 1)], BF16)
            nc.vector.memset(v_aug, 1.0)

            def emit_tok_dma(ti):
                """tokens DMA (scalar ring) + bf16 cast (DVE)."""
                tok_nat = pb_tok.tile([P, TB, HID], F32, tag="tok")
                nc.scalar.dma_start(
                    out=tok_nat,
                    in_=tokens[ti * TC:(ti + 1) * TC, :].rearrange(
                        "(n p) h -> p n h", p=P
                    ),
                )
                tok_bf = pb_tokb.tile([P, TB, HID], BF16, tag="tokb")
                nc.vector.tensor_copy(out=tok_bf, in_=tok_nat)
                return tok_bf

            def emit_tok_xbar(tok_bf):
                """xbar transpose (scalar ring):
                tokt[p, tb, hc, t2] = tokens[.., tb*128 + t2, hc*128 + p]"""
                tokt = pb_tokt.tile([P, TB, HC, P], BF16, tag="tokt")
                nc.scalar.dma_start_transpose(out=tokt, in_=tok_bf)
                return tokt

            def emit_scores_range(ti, tokt, pts, sb_lo, sb_hi):
                # scores^T [s, t] = CAT^T @ tokens^T -> exp -> P^T tiles
                for sb in range(sb_lo, sb_hi):
                    pscore = ps_s.tile([P, TC], F32, tag="s")
                    for hc in range(HC):
                        nc.tensor.matmul(
                            pscore,
                            cat_sb[:, hc, sb * P:(sb + 1) * P],
                            tokt[:, :, hc, :],
                            start=(hc == 0),
                            stop=(hc == HC - 1),
                        )
                    pt_tile = pb_pt.tile([P, TC], BF16, tag="pt")
                    nc.scalar.activation(
                        out=pt_tile,
                        in_=pscore,
                        func=mybir.ActivationFunctionType.Exp,
                        scale=inv_sqrt_emb,
                    )
                    pts.append(pt_tile)

            # ---- Phase A: context -> context^T -> K^T -> CAT, V ----
            with (
                tc.tile_pool(name="pa_sbuf", bufs=4) as pa_sbuf,
                tc.tile_pool(name="pa_wst", bufs=1) as pa_wst,
                tc.tile_pool(name="pa_wb", bufs=1) as pa_wb,
                tc.tile_pool(name="pa_ctxt", bufs=1) as pa_ctxt,
                tc.tile_pool(name="pa_psum", bufs=2, space="PSUM") as pa_psum,
                tc.tile_pool(name="pa_psum_kv", bufs=2, space="PSUM") as pa_psum_kv,
            ):
                # Input DMA triggers. Rings are FIFO at ~155 GB/s each, so
                # the critical tensors lead: K^T (s-half 0) needs ctx q0/q1
                # + Wk first.
                ctx_nat = [None] * NQ
                for q in range(NQ):
                    ctx_nat[q] = pa_sbuf.tile(
                        [P, QS, CTX], F32, tag="ctxn", name=f"ctx_nat{q}"
                    )

                def dma_ctx(q, eng):
                    eng.dma_start(
                        out=ctx_nat[q],
                        in_=context[q * (S // NQ):(q + 1) * (S // NQ), :].rearrange(
                            "(n p) c -> p n c", p=P
                        ),
                    )

                wk_st = pa_wst.tile([P, CC, EMB], F32, tag="wkst")
                wq_st = pa_wst.tile([P, HC, EMB], F32, tag="wqst")
                wv_st = pa_wst.tile([P, CC, HID], F32, tag="wvst")
                dma_ctx(0, nc.sync)
                dma_ctx(1, nc.scalar)
                nc.sync.dma_start(out=wk_st, in_=wk.rearrange("(c p) e -> p c e", p=P))
                nc.scalar.dma_start(out=wq_st, in_=wq.rearrange("(h p) e -> p h e", p=P))
                dma_ctx(2, nc.sync)
                dma_ctx(3, nc.scalar)
                nc.scalar.dma_start(out=wv_st, in_=wv.rearrange("(c p) h -> p c h", p=P))

                # bf16 casts: ctx half 0 on DVE, Wk/Wq on Scalar
                ctx_bf = []
                for q in range(2):
                    cb = pa_sbuf.tile(
                        [P, QS, CTX], BF16, tag="ctxb", name=f"ctx_bf{q}"
                    )
                    nc.vector.tensor_copy(out=cb, in_=ctx_nat[q])
                    ctx_bf.append(cb)
                wk_sb = pa_wb.tile([P, CC, EMB], BF16)
                nc.scalar.activation(
                    out=wk_sb, in_=wk_st, func=mybir.ActivationFunctionType.Copy
                )
                wq_sb = pa_wb.tile([P, HC, EMB], BF16)
                nc.scalar.activation(
                    out=wq_sb, in_=wq_st, func=mybir.ActivationFunctionType.Copy
                )

                ctxt = pa_ctxt.tile([P, CC, S], BF16)   # context^T [c, s]
                # Wq^T via xbar: wqt[p, hc, ec, c] = Wq^T[ec*128+p, hc*128+c]
                wqt = pa_wb.tile([P, HC, EC, P], BF16)
                nc.sync.dma_start_transpose(out=wqt, in_=wq_sb)

                kt = pa_ctxt.tile([P, EC, S], BF16)     # K^T [e, s]

                def t_ctx_half(half):
                    # PE transpose-mode; one psum tile per (cc, half)
                    for cc in range(CC):
                        pt = pa_psum.tile([P, 512], BF16, tag="pa_t")
                        for j in range(4):
                            q, jj = half * 2 + j // 2, j % 2
                            nc.tensor.transpose(
                                pt[:, j * P:(j + 1) * P],
                                ctx_bf[q][:, jj, cc * P:(cc + 1) * P],
                                ident,
                            )
                        nc.vector.tensor_copy(
                            out=ctxt[:, cc, half * 512:(half + 1) * 512], in_=pt
                        )

                def mm_kt(sh):
                    # K^T[e, s-half] accumulation over c chunks
                    for ec in range(EC):
                        pk = pa_psum_kv.tile([P, 512], F32, tag="pa_kv")
                        for cc in range(CC):
                            nc.tensor.matmul(
                                pk,
                                wk_sb[:, cc, ec * P:(ec + 1) * P],
                                ctxt[:, cc, sh * 512:(sh + 1) * 512],
                                start=(cc == 0),
                                stop=(cc == CC - 1),
                            )
                        nc.vector.tensor_copy(
                            out=kt[:, ec, sh * 512:(sh + 1) * 512], in_=pk
                        )

                def mm_cat(sh):
                    # CAT[h, s-half] = Wq @ K^T, accumulation over e chunks
                    for hc in range(HC):
                        pc = pa_psum_kv.tile([P, 512], F32, tag="pa_kv")
                        for ec in range(EC):
                            nc.tensor.matmul(
                                pc,
                                wqt[:, hc, ec, :],
                                kt[:, ec, sh * 512:(sh + 1) * 512],
                                start=(ec == 0),
                                stop=(ec == EC - 1),
                            )
                        nc.vector.tensor_copy(
                            out=cat_sb[:, hc, sh * 512:(sh + 1) * 512], in_=pc
                        )

                def mm_v(sb_lo, sb_hi):
                    # V[s, h] accumulation over c chunks -> augmented layout
                    for sb in range(sb_lo, sb_hi):
                        pv = pa_psum_kv.tile([P, 512], F32, tag="pa_kv")
                        for cc in range(CC):
                            nc.tensor.matmul(
                                pv,
                                ctxt[:, cc, sb * P:(sb + 1) * P],
                                wv_sb[:, cc, :],
                                start=(cc == 0),
                                stop=(cc == CC - 1),
                            )
                        nc.vector.tensor_copy(
                            out=v_aug[:, sb, 0:HH], in_=pv[:, 0:HH]
                        )
                        nc.vector.tensor_copy(
                            out=v_aug[:, sb, HH + 1:2 * HH + 1], in_=pv[:, HH:HID]
                        )

                t_ctx_half(0)
                mm_kt(0)
                # JIT: ctx half-1 casts (DVE queue served half-0 copies
                # first), wv cast, chunk-0 token pipeline
                for q in (2, 3):
                    cb = pa_sbuf.tile(
                        [P, QS, CTX], BF16, tag="ctxb", name=f"ctx_bf{q}"
                    )
                    nc.vector.tensor_copy(out=cb, in_=ctx_nat[q])
                    ctx_bf.append(cb)
                wv_sb = pa_wb.tile([P, CC, HID], BF16)
                nc.scalar.activation(
                    out=wv_sb, in_=wv_st, func=mybir.ActivationFunctionType.Copy
                )
                tok_bf0 = emit_tok_dma(0)
                tokt0 = emit_tok_xbar(tok_bf0)
                mm_cat(0)
                mm_v(0, 4)
                # chunk-0 scores fill the PE while the half-1 DMAs stream
                pts0 = []
                emit_scores_range(0, tokt0, pts0, 0, 4)
                t_ctx_half(1)
                mm_kt(1)
                mm_cat(1)
                emit_scores_range(0, tokt0, pts0, 4, 8)
                mm_v(4, 8)

            # ---- Phase B: stream over t chunks ----
            with tc.tile_pool(name="ps_ctx", bufs=2, space="PSUM") as ps_ctx:
                def emit_attn(ti, pts):
                    # attn@V with the ones-augmented V: two N=257 matmuls per
                    # s-block into two PSUM banks; column 256 of bank a is the
                    # softmax denominator in [t-part, 1] layout.
                    o_all = pb_out.tile([P, TB, HID], F32, tag="out")
                    for tb in range(TB):
                        pca = ps_ctx.tile([P, HH + 1], F32, tag="ctxa")
                        pcb = ps_ctx.tile([P, HH + 1], F32, tag="ctxb")
                        for sb in range(SB):
                            st = pts[sb][:, tb * P:(tb + 1) * P]
                            nc.tensor.matmul(
                                pca,
                                st,
                                v_aug[:, sb, 0:HH + 1],
                                start=(sb == 0),
                                stop=(sb == SB - 1),
                            )
                            nc.tensor.matmul(
                                pcb,
                                st,
                                v_aug[:, sb, HH + 1:2 * (HH + 1)],
                                start=(sb == 0),
                                stop=(sb == SB - 1),
                            )
                        rec = pb_small.tile([P, 1], F32, tag="rec")
                        nc.vector.reciprocal(out=rec, in_=pca[:, HH:HH + 1])
                        nc.vector.tensor_scalar_mul(
                            o_all[:, tb, 0:HH], pca[:, 0:HH], rec
                        )
                        nc.vector.tensor_scalar_mul(
                            o_all[:, tb, HH:HID], pcb[:, 0:HH], rec
                        )
                        nc.sync.dma_start(
                            out=out[ti * TC + tb * P:ti * TC + (tb + 1) * P, :],
                            in_=o_all[:, tb, :],
                        )

                pts = pts0
                tok_bf_next = emit_tok_dma(1)
                for ti in range(N_TC):
                    # next chunk's xbar fires only after this chunk's exps
                    # are queued, so it never blocks them on the scalar queue
                    if ti + 1 < N_TC:
                        tokt_next = emit_tok_xbar(tok_bf_next)
                    if ti + 2 < N_TC:
                        tok_bf_next = emit_tok_dma(ti + 2)
                    emit_attn(ti, pts)
                    if ti + 1 < N_TC:
                        pts = []
                        emit_scores_range(ti + 1, tokt_next, pts, 0, SB)

    nc.compile()
    return nc


_NC_CACHE = None


def _get_nc():
    global _NC_CACHE
    if _NC_CACHE is None:
        _NC_CACHE = build()
    return _NC_CACHE


def kernel(tokens, context, Wq, Wk, Wv):
    tokens = np.ascontiguousarray(np.asarray(tokens, dtype=np.float32))
    context = np.ascontiguousarray(np.asarray(context, dtype=np.float32))
    Wq = np.ascontiguousarray(np.asarray(Wq, dtype=np.float32))
    Wk = np.ascontiguousarray(np.asarray(Wk, dtype=np.float32))
    Wv = np.ascontiguousarray(np.asarray(Wv, dtype=np.float32))

    nc = _get_nc()
    in_maps = [
        {
            "tokens": tokens[b],
            "context": context[b],
            "Wq": Wq,
            "Wk": Wk,
            "Wv": Wv,
        }
        for b in range(B)
    ]
    res = run_bass_kernel_spmd(nc, in_maps, core_ids=list(range(B)))
    return np.stack([res.results[b]["out"] for b in range(B)], axis=0)


# revision 28
# speedup vs baseline: 1.1382x; 1.0168x over previous
"""Cross-attention Bass/Tile kernel for Trainium2, data-parallel over batch on
8 NeuronCores.

Reference computation (per batch b):
    Q = tokens @ Wq            [T, EMB]
    K = context @ Wk           [S, EMB]
    V = context @ Wv           [S, HID]
    scores = Q @ K.T / sqrt(EMB)
    attn = softmax(scores, axis=-1)
    out = attn @ V             [T, HID]

Shapes: B=8, T=4096, S=1024, HID=512, EMB=512, CTX=768 (fp32).

Design notes:
- One batch per core (B == n_cores == 8), no collectives.
- Weight fold: scores = tokens @ (Wq @ K^T), so the per-token Q projection is
  eliminated. Phase A computes CAT = Wq @ K^T [h, s] once per batch (K^T =
  Wk^T @ context^T); phase B contracts tokens^T against CAT directly.
- Scores are computed TRANSPOSED, [s, t], so the exp(P^T) tiles in SBUF feed
  the attn@V matmul directly as the stationary operand — no transpose of the
  4M-element P.
- Softmax skips the max-subtraction: scores/sqrt(EMB) are ~N(0,1) here; the
  1/sqrt(EMB) is folded into the ACT exp scale.
- All matmul operands are bf16 (PSUM accumulation stays fp32). fp32->bf16
  input casts run on DVE (ctx, tokens) and Scalar (weights); exp runs on the
  Scalar engine.
- tokens^T and Wq^T are produced by single-trigger DMA xbar transposes
  (2-byte SBUF->SBUF; for out [128, M, 128]: out[p, m, c] = in[c, m*128+p]).
  context^T stays on PE transpose-mode: the xbar rings are FIFO with only
  ~155 GB/s each, so phase-A xbar transfers would queue behind the bulk
  input DMAs on the head critical path.
- Moving operands must keep >=1024B-aligned strides: a [128-elem x
  stride-768-elem] bf16 pattern runs at half rate; stride-512-elem (1024B)
  runs at full rate (tokt layout relies on this).
- Row sums of exp are folded into the attn@V matmul: V is augmented with a
  ones column (layout [V[:, :256] | 1 | V[:, 256:] | 1]) and each t-block
  runs 2x N=257 matmuls per s-block into two PSUM banks. Column 256 of the
  first bank is sum_s exp = the softmax denominator, in exactly the layout
  ([t-part, 1]) needed for the per-partition reciprocal + scale.
- Token loads are software-pipelined one chunk ahead (DMA on the scalar
  ring -> DVE cast -> xbar transpose, with the xbar trigger emitted after
  the chunk's exps so it never blocks them); chunk 0 is hoisted into phase
  A, and chunk 0's scores for s-half 0/1 are interleaved into phase A's
  DMA shadow.
- Input DMAs ride the two HWDGE rings; the SDMA engines drain rings at
  ~155 GB/s each, FIFO per ring, so the critical tensors lead each ring:
  sync: [ctx q0, Wk, ctx q2, out-stores]; scalar: [ctx q1, Wq, ctx q3, Wv,
  tokens].
"""

import math

import numpy as np

from concourse import bacc, mybir, tile
from concourse.bass_utils import run_bass_kernel_spmd
from concourse.masks import make_identity

B, T, S = 8, 4096, 1024
HID, EMB, CTX = 512, 512, 768
P = 128  # partitions
TC = 512  # t-chunk processed per phase-B iteration
N_TC = T // TC  # 8
F32 = mybir.dt.float32
BF16 = mybir.dt.bfloat16

HC = HID // P  # 4 h chunks
EC = EMB // P  # 4 e chunks
CC = CTX // P  # 6 c chunks
SB = S // P    # 8 s blocks
TB = TC // P   # 4 t blocks per chunk
HH = HID // 2  # 256, half of the output features per augmented-V matmul
NQ = 4         # context DMA quarters
QS = SB // NQ  # 2 s-blocks per quarter


def build():
    nc = bacc.Bacc("TRN2", target_bir_lowering=False, debug=False)

    tokens = nc.declare_dram_parameter("tokens", [T, HID], F32, isOutput=False)
    context = nc.declare_dram_parameter("context", [S, CTX], F32, isOutput=False)
    wq = nc.declare_dram_parameter("Wq", [HID, EMB], F32, isOutput=False)
    wk = nc.declare_dram_parameter("Wk", [CTX, EMB], F32, isOutput=False)
    wv = nc.declare_dram_parameter("Wv", [CTX, HID], F32, isOutput=False)
    out = nc.declare_dram_parameter("out", [T, HID], F32, isOutput=True)

    inv_sqrt_emb = 1.0 / math.sqrt(EMB)

    with tile.TileContext(nc) as tc:
        with (
            tc.tile_pool(name="persist", bufs=1) as persist,
            tc.tile_pool(name="pb_tok", bufs=2) as pb_tok,
            tc.tile_pool(name="pb_tokb", bufs=2) as pb_tokb,
            tc.tile_pool(name="pb_tokt", bufs=2) as pb_tokt,
            tc.tile_pool(name="pb_pt", bufs=16) as pb_pt,
            tc.tile_pool(name="pb_small", bufs=8) as pb_small,
            tc.tile_pool(name="pb_out", bufs=2) as pb_out,
            tc.tile_pool(name="ps_s", bufs=4, space="PSUM") as ps_s,
        ):
            ident = persist.tile([P, P], BF16)
            make_identity(nc, ident)

            # CAT = Wq @ K^T [h, s], built once per batch
            cat_sb = persist.tile([P, HC, S], BF16)
            # V augmented with ones columns: [V[:, 0:256] | 1 | V[:, 256:512] | 1]
            v_aug = persist.tile([P, SB, 2 * (HH + 1)], BF16)
            nc.vector.memset(v_aug, 1.0)

            def emit_tok_dma(ti):
                """tokens DMA (scalar ring) + bf16 cast (DVE)."""
                tok_nat = pb_tok.tile([P, TB, HID], F32, tag="tok")
                nc.scalar.dma_start(
                    out=tok_nat,
                    in_=tokens[ti * TC:(ti + 1) * TC, :].rearrange(
                        "(n p) h -> p n h", p=P
                    ),
                )
                tok_bf = pb_tokb.tile([P, TB, HID], BF16, tag="tokb")
                nc.vector.tensor_copy(out=tok_bf, in_=tok_nat)
                return tok_bf

            def emit_tok_xbar(tok_bf):
                """xbar transpose (scalar ring):
                tokt[p, tb, hc, t2] = tokens[.., tb*128 + t2, hc*128 + p]"""
                tokt = pb_tokt.tile([P, TB, HC, P], BF16, tag="tokt")
                nc.scalar.dma_start_transpose(out=tokt, in_=tok_bf)
                return tokt

            def emit_scores_range(ti, tokt, pts, sb_lo, sb_hi):
                # scores^T [s, t] = CAT^T @ tokens^T -> exp -> P^T tiles
                for sb in range(sb_lo, sb_hi):
                    pscore = ps_s.tile([P, TC], F32, tag="s")
                    for hc in range(HC):
                        nc.tensor.matmul(
                            pscore,
                            cat_sb[:, hc, sb * P:(sb + 1) * P],
                            tokt[:, :, hc, :],
                            start=(hc == 0),
                            stop=(hc == HC - 1),
                        )
                    pt_tile = pb_pt.tile([P, TC], BF16, tag="pt")
                    nc.scalar.activation(
                        out=pt_tile,
                        in_=pscore,
                        func=mybir.ActivationFunctionType.Exp,
                        scale=inv_sqrt_emb,
                    )
                    pts.append(pt_tile)

            # ---- Phase A: context -> context^T -> K^T -> CAT, V ----
            with (
                tc.tile_pool(name="pa_sbuf", bufs=2) as pa_sbuf,
                tc.tile_pool(name="pa_wst", bufs=1) as pa_wst,
                tc.tile_pool(name="pa_wb", bufs=1) as pa_wb,
                tc.tile_pool(name="pa_ctxt", bufs=1) as pa_ctxt,
                tc.tile_pool(name="pa_psum", bufs=2, space="PSUM") as pa_psum,
                tc.tile_pool(name="pa_psum_kv", bufs=2, space="PSUM") as pa_psum_kv,
            ):
                # Input DMA triggers. Rings are FIFO at ~155 GB/s each, so
                # the critical tensors lead: K^T (s-half 0) needs ctx q0/q1
                # + Wk first.
                ctx_nat = [None] * NQ
                for q in range(NQ):
                    ctx_nat[q] = pa_sbuf.tile(
                        [P, QS, CTX], F32, tag="ctxn", name=f"ctx_nat{q}"
                    )

                def dma_ctx(q, eng):
                    eng.dma_start(
                        out=ctx_nat[q],
                        in_=context[q * (S // NQ):(q + 1) * (S // NQ), :].rearrange(
                            "(n p) c -> p n c", p=P
                        ),
                    )

                # wave 1: ctx half 0 + Wk + Wq only — the SDMA engines drain
                # ALL in-flight transfers fair-share, so wave-2 triggers are
                # gated (pool-buffer reuse: q2/q3 reuse q0/q1's staging, Wv
                # reuses Wk's) to keep wave-1 latency minimal.
                wk_st = pa_wst.tile([P, CC, EMB], F32, tag="w6")
                wq_st = pa_wst.tile([P, HC, EMB], F32, tag="w4")
                dma_ctx(0, nc.sync)
                dma_ctx(1, nc.scalar)
                nc.sync.dma_start(out=wk_st, in_=wk.rearrange("(c p) e -> p c e", p=P))
                nc.scalar.dma_start(out=wq_st, in_=wq.rearrange("(h p) e -> p h e", p=P))

                # bf16 casts: ctx half 0 on DVE, Wk/Wq on Scalar
                ctx_bf = []
                for q in range(2):
                    cb = pa_sbuf.tile(
                        [P, QS, CTX], BF16, tag="ctxb", name=f"ctx_bf{q}"
                    )
                    nc.vector.tensor_copy(out=cb, in_=ctx_nat[q])
                    ctx_bf.append(cb)
                # wave 2: ctx half 1 (gated on the half-0 casts via ctxn
                # buffer reuse)
                dma_ctx(2, nc.sync)
                dma_ctx(3, nc.scalar)
                wk_sb = pa_wb.tile([P, CC, EMB], BF16)
                nc.scalar.activation(
                    out=wk_sb, in_=wk_st, func=mybir.ActivationFunctionType.Copy
                )
                wq_sb = pa_wb.tile([P, HC, EMB], BF16)
                nc.scalar.activation(
                    out=wq_sb, in_=wq_st, func=mybir.ActivationFunctionType.Copy
                )
                # wave 3: Wv (reuses Wk's staging buffer -> waits wk cast;
                # emitted after the casts so the scalar queue can't deadlock)
                wv_st = pa_wst.tile([P, CC, HID], F32, tag="w6")
                nc.scalar.dma_start(out=wv_st, in_=wv.rearrange("(c p) h -> p c h", p=P))

                ctxt = pa_ctxt.tile([P, CC, S], BF16)   # context^T [c, s]
                # Wq^T via xbar: wqt[p, hc, ec, c] = Wq^T[ec*128+p, hc*128+c]
                wqt = pa_wb.tile([P, HC, EC, P], BF16)
                nc.sync.dma_start_transpose(out=wqt, in_=wq_sb)

                kt = pa_ctxt.tile([P, EC, S], BF16)     # K^T [e, s]

                def t_ctx_half(half):
                    # PE transpose-mode; one psum tile per (cc, half)
                    for cc in range(CC):
                        pt = pa_psum.tile([P, 512], BF16, tag="pa_t")
                        for j in range(4):
                            q, jj = half * 2 + j // 2, j % 2
                            nc.tensor.transpose(
                                pt[:, j * P:(j + 1) * P],
                                ctx_bf[q][:, jj, cc * P:(cc + 1) * P],
                                ident,
                            )
                        nc.vector.tensor_copy(
                            out=ctxt[:, cc, half * 512:(half + 1) * 512], in_=pt
                        )

                def mm_kt(sh):
                    # K^T[e, s-half] accumulation over c chunks
                    for ec in range(EC):
                        pk = pa_psum_kv.tile([P, 512], F32, tag="pa_kv")
                        for cc in range(CC):
                            nc.tensor.matmul(
                                pk,
                                wk_sb[:, cc, ec * P:(ec + 1) * P],
                                ctxt[:, cc, sh * 512:(sh + 1) * 512],
                                start=(cc == 0),
                                stop=(cc == CC - 1),
                            )
                        nc.vector.tensor_copy(
                            out=kt[:, ec, sh * 512:(sh + 1) * 512], in_=pk
                        )

                def mm_cat(sh):
                    # CAT[h, s-half] = Wq @ K^T, accumulation over e chunks
                    for hc in range(HC):
                        pc = pa_psum_kv.tile([P, 512], F32, tag="pa_kv")
                        for ec in range(EC):
                            nc.tensor.matmul(
                                pc,
                                wqt[:, hc, ec, :],
                                kt[:, ec, sh * 512:(sh + 1) * 512],
                                start=(ec == 0),
                                stop=(ec == EC - 1),
                            )
                        nc.vector.tensor_copy(
                            out=cat_sb[:, hc, sh * 512:(sh + 1) * 512], in_=pc
                        )

                def mm_v(sb_lo, sb_hi):
                    # V[s, h] accumulation over c chunks -> augmented layout
                    for sb in range(sb_lo, sb_hi):
                        pv = pa_psum_kv.tile([P, 512], F32, tag="pa_kv")
                        for cc in range(CC):
                            nc.tensor.matmul(
                                pv,
                                ctxt[:, cc, sb * P:(sb + 1) * P],
                                wv_sb[:, cc, :],
                                start=(cc == 0),
                                stop=(cc == CC - 1),
                            )
                        nc.vector.tensor_copy(
                            out=v_aug[:, sb, 0:HH], in_=pv[:, 0:HH]
                        )
                        nc.vector.tensor_copy(
                            out=v_aug[:, sb, HH + 1:2 * HH + 1], in_=pv[:, HH:HID]
                        )

                t_ctx_half(0)
                mm_kt(0)
                # JIT: ctx half-1 casts (DVE queue served half-0 copies
                # first), wv cast, chunk-0 token pipeline
                for q in (2, 3):
                    cb = pa_sbuf.tile(
                        [P, QS, CTX], BF16, tag="ctxb", name=f"ctx_bf{q}"
                    )
                    nc.vector.tensor_copy(out=cb, in_=ctx_nat[q])
                    ctx_bf.append(cb)
                wv_sb = pa_wb.tile([P, CC, HID], BF16)
                nc.scalar.activation(
                    out=wv_sb, in_=wv_st, func=mybir.ActivationFunctionType.Copy
                )
                tok_bf0 = emit_tok_dma(0)
                tokt0 = emit_tok_xbar(tok_bf0)
                mm_cat(0)
                mm_v(0, 4)
                # chunk-0 scores fill the PE while the half-1 DMAs stream
                pts0 = []
                emit_scores_range(0, tokt0, pts0, 0, 4)
                t_ctx_half(1)
                mm_kt(1)
                mm_cat(1)
                emit_scores_range(0, tokt0, pts0, 4, 8)
                mm_v(4, 8)

            # ---- Phase B: stream over t chunks ----
            with tc.tile_pool(name="ps_ctx", bufs=2, space="PSUM") as ps_ctx:
                def emit_attn(ti, pts):
                    # attn@V with the ones-augmented V: two N=257 matmuls per
                    # s-block into two PSUM banks; column 256 of bank a is the
                    # softmax denominator in [t-part, 1] layout.
                    o_all = pb_out.tile([P, TB, HID], F32, tag="out")
                    for tb in range(TB):
                        pca = ps_ctx.tile([P, HH + 1], F32, tag="ctxa")
                        pcb = ps_ctx.tile([P, HH + 1], F32, tag="ctxb")
                        for sb in range(SB):
                            st = pts[sb][:, tb * P:(tb + 1) * P]
                            nc.tensor.matmul(
                                pca,
                                st,
                                v_aug[:, sb, 0:HH + 1],
                                start=(sb == 0),
                                stop=(sb == SB - 1),
                            )
                            nc.tensor.matmul(
                                pcb,
                                st,
                                v_aug[:, sb, HH + 1:2 * (HH + 1)],
                                start=(sb == 0),
                                stop=(sb == SB - 1),
                            )
                        rec = pb_small.tile([P, 1], F32, tag="rec")
                        nc.vector.reciprocal(out=rec, in_=pca[:, HH:HH + 1])
                        nc.vector.tensor_scalar_mul(
                            o_all[:, tb, 0:HH], pca[:, 0:HH], rec
                        )
                        nc.vector.tensor_scalar_mul(
                            o_all[:, tb, HH:HID], pcb[:, 0:HH], rec
                        )
                        nc.sync.dma_start(
                            out=out[ti * TC + tb * P:ti * TC + (tb + 1) * P, :],
                            in_=o_all[:, tb, :],
                        )

                pts = pts0
                tok_bf_next = emit_tok_dma(1)
                for ti in range(N_TC):
                    # next chunk's xbar fires only after this chunk's exps
                    # are queued, so it never blocks them on the scalar queue
                    if ti + 1 < N_TC:
                        tokt_next = emit_tok_xbar(tok_bf_next)
                    if ti + 2 < N_TC:
                        tok_bf_next = emit_tok_dma(ti + 2)
                    emit_attn(ti, pts)
                    if ti + 1 < N_TC:
                        pts = []
                        emit_scores_range(ti + 1, tokt_next, pts, 0, SB)

    nc.compile()
    return nc


_NC_CACHE = None


def _get_nc():
    global _NC_CACHE
    if _NC_CACHE is None:
        _NC_CACHE = build()
    return _NC_CACHE


def kernel(tokens, context, Wq, Wk, Wv):
    tokens = np.ascontiguousarray(np.asarray(tokens, dtype=np.float32))
    context = np.ascontiguousarray(np.asarray(context, dtype=np.float32))
    Wq = np.ascontiguousarray(np.asarray(Wq, dtype=np.float32))
    Wk = np.ascontiguousarray(np.asarray(Wk, dtype=np.float32))
    Wv = np.ascontiguousarray(np.asarray(Wv, dtype=np.float32))

    nc = _get_nc()
    in_maps = [
        {
            "tokens": tokens[b],
            "context": context[b],
            "Wq": Wq,
            "Wk": Wk,
            "Wv": Wv,
        }
        for b in range(B)
    ]
    res = run_bass_kernel_spmd(nc, in_maps, core_ids=list(range(B)))
    return np.stack([res.results[b]["out"] for b in range(B)], axis=0)
